# revision 15
# baseline (speedup 1.0000x reference)
"""Trainium2 Bass kernel for nn_LpAlignEntropyLoss.

Loss over three views z1,z2,z3 (each [8192,128] f32):
  for each pair (i<j):
    pos += mean_m ||zi_m - zj_m||
    neg += mean_m [ ln(sum_n exp(-d_mn)) - ln(B) ],  d = cdist(zi, zj)
  loss = (0.5*pos + 0.5*neg) / 3

Strategy: shard the 8192 rows across 8 cores (1024 each). Each core holds
all three views transposed ([128=D, 8192=B]) in SBUF as bf16, computes its
row-block of each pairwise squared-distance matrix with PE matmuls: the
-2*dot term is the main bf16 matmul, the +b2[n] column-norm term is folded
in as a K=1 accumulating matmul, and the +a2[m] row-norm term rides the
ACT bias. ScalarE then does sqrt (PSUM->SBUF fp16) and exp(16-d) with a
fused row-accumulate, batched per m-block by activation-table set to
bound table-switch cost. Host sums the 8 partial scalars; no collectives.
"""

import math

import numpy as np
import ml_dtypes

import concourse.bacc as bacc
import concourse.bass as bass
import concourse.mybir as mybir
import concourse.tile as tile
from concourse.tile import add_dep_helper
from concourse.bass_utils import run_bass_kernel_spmd

B, D = 8192, 128
NCORES = 8
ML = B // NCORES          # rows per core (1024)
MB = ML // 128            # m-blocks per core (8)
NCHUNK = 2048             # psum chunk (4 banks)
NQ = B // NCHUNK          # chunks per row (4)
PAIRS = [(0, 1), (0, 2), (1, 2)]
TAU = 1.0
ALPHA = 0.5
B2_CENTER = 128.0         # E[||z||^2] for z~N(0,I_128); centers the bf16 bias row
EXP_SHIFT = 16.0          # e^(SHIFT-d) keeps fp16 in range for d in [9, 27]

F32 = mybir.dt.float32
BF16 = mybir.dt.bfloat16
FP16 = mybir.dt.float16
AF = mybir.ActivationFunctionType
ALU = mybir.AluOpType
AX = mybir.AxisListType

# True : d = Sqrt(sq) (sqrt table <-> exp table, switched per m-block)
# False: d = Exp(0.5*Ln(sq)) (single natural_log_exp table, 1 extra pass)
SQRT_MODE = True


def build(nc: bacc.Bacc):
    zt = [nc.dram_tensor(f"zt{v}", [D, B], BF16, kind="ExternalInput") for v in range(3)]
    blk = [nc.dram_tensor(f"blk{v}", [D, ML], BF16, kind="ExternalInput") for v in range(3)]
    out = nc.dram_tensor("out", [2, 1], F32, kind="ExternalOutput")

    rhs_views = sorted({j for _, j in PAIRS})  # views used as columns (1, 2)

    with tile.TileContext(nc) as tc:
        with tc.tile_pool(name="persist", bufs=1) as persist:
            # ---- persistent SBUF ----
            rhs_views_l = sorted({j for _, j in PAIRS})
            ztc = {j: [persist.tile([D, NCHUNK], BF16, tag=f"ztc{j}_{q}", name=f"ztc{j}_{q}")
                       for q in range(NQ)] for j in rhs_views_l}
            blks = [persist.tile([D, ML], BF16, tag=f"blks{v}", name=f"blks{v}") for v in range(3)]
            for j in rhs_views_l:
                nc.sync.dma_start(ztc[j][0][:], zt[j][:, 0:NCHUNK])
            for v in range(3):
                nc.sync.dma_start(blks[v][:], blk[v][:])
            for q in range(1, NQ):
                for j in rhs_views_l:
                    nc.sync.dma_start(ztc[j][q][:], zt[j][:, q * NCHUNK:(q + 1) * NCHUNK])

            ones_bf_row = persist.tile([1, 128], BF16, tag="ones_bf_row")
            nc.vector.memset(ones_bf_row[:], 1.0)
            ones_bf_col = persist.tile([128, 1], BF16, tag="ones_bf_col")
            nc.vector.memset(ones_bf_col[:], 1.0)
            ones_f32_col = persist.tile([128, 1], F32, tag="ones_f32_col")
            nc.vector.memset(ones_f32_col[:], 1.0)
            shift16 = persist.tile([128, 1], F32, tag="shift16")
            nc.vector.memset(shift16[:], EXP_SHIFT)

            # b2half[j][n] = -0.5*(||z_j[n]||^2 - B2_CENTER), bf16 row
            b2half = {j: persist.tile([1, B], BF16, tag=f"b2h{j}", name=f"b2h{j}") for j in rhs_views}
            # a2sb[v] = per-row-block norms ||z_v[m]||^2, [128, MB] f32
            a2sb = [persist.tile([128, MB], F32, tag=f"a2{v}", name=f"a2{v}") for v in range(3)]
            # biasp[p] = a2_i[m] + B2_CENTER for pair p (ACT bias columns)
            biasp = [persist.tile([128, MB], F32, tag=f"biasp{p}", name=f"biasp{p}") for p in range(3)]
            # ipsb[p] = <zi_m, zj_m> for own rows, [128, MB] f32
            ipsb = [persist.tile([128, MB], F32, tag=f"ip{p}", name=f"ip{p}") for p in range(3)]
            # sum_n exp(SHIFT - d) accumulators, one col per (pair, m-block)
            sacc = persist.tile([128, 3 * MB], F32, tag="sacc")
            # positive-pair squared distances (cols per pair) and their sqrts
            sqpos = persist.tile([128, 3 * MB], F32, tag="sqpos")
            dpos = persist.tile([128, 3 * MB], F32, tag="dpos")

            # ---- prep phase ----
            with (
                tc.tile_pool(name="prep", bufs=2) as prep,
                tc.tile_pool(name="ppsum", bufs=1, space="PSUM") as ppsum,
                tc.tile_pool(name="ppsum2", bufs=2, space="PSUM") as ppsum2,
            ):
                # column norms b2 (ones-matmul over squared columns)
                for q in range(NQ):
                    for j in rhs_views:
                        sq = prep.tile([D, NCHUNK], BF16, tag="sqc", name="sqc")
                        nc.vector.tensor_mul(sq[:], ztc[j][q][:], ztc[j][q][:])
                        pb = ppsum.tile([1, NCHUNK], F32, tag="pb", name="pb")
                        for s in range(NCHUNK // 512):
                            nc.tensor.matmul(pb[0:1, s * 512:(s + 1) * 512],
                                             ones_bf_col[:], sq[:, s * 512:(s + 1) * 512],
                                             start=True, stop=True)
                        nc.vector.tensor_scalar(b2half[j][0:1, q * NCHUNK:(q + 1) * NCHUNK],
                                                pb[0:1, :], -0.5, 0.5 * B2_CENTER,
                                                ALU.mult, ALU.add)

                # row norms a2 and positive-pair dots ip (ones-matmuls)
                blksq = [prep.tile([D, ML], F32, tag=f"blksq{v}", name=f"blksq{v}") for v in range(3)]
                for v in range(3):
                    nc.vector.tensor_mul(blksq[v][:], blks[v][:], blks[v][:])
                    for k in range(MB):
                        pa = ppsum2.tile([128, 1], F32, tag="pa", name="pa")
                        nc.tensor.matmul(pa[:], blksq[v][:, k * 128:(k + 1) * 128],
                                         ones_f32_col[:], start=True, stop=True)
                        nc.vector.tensor_copy(a2sb[v][:, k:k + 1], pa[:])
                for p, (i, j) in enumerate(PAIRS):
                    ipf = prep.tile([D, ML], F32, tag="ipf", name="ipf")
                    nc.vector.tensor_mul(ipf[:], blks[i][:], blks[j][:])
                    for k in range(MB):
                        pa = ppsum2.tile([128, 1], F32, tag="pa", name="pa")
                        nc.tensor.matmul(pa[:], ipf[:, k * 128:(k + 1) * 128],
                                         ones_f32_col[:], start=True, stop=True)
                        nc.vector.tensor_copy(ipsb[p][:, k:k + 1], pa[:])
                    nc.vector.tensor_scalar_add(biasp[p][:], a2sb[i][:], B2_CENTER)
                    apre = prep.tile([128, MB], F32, tag="apre", name="apre")
                    nc.vector.tensor_add(apre[:], a2sb[i][:], a2sb[j][:])
                    nc.vector.scalar_tensor_tensor(sqpos[:, p * MB:(p + 1) * MB],
                                                   ipsb[p][:], -2.0, apre[:],
                                                   ALU.mult, ALU.add)

            # ---- main loop ----
            with (
                tc.tile_pool(name="mpsum", bufs=2, space="PSUM") as mpsum,
                tc.tile_pool(name="dtiles", bufs=6) as dpool,
                tc.tile_pool(name="ttiles", bufs=2) as tpool,
            ):
                prev_act = None

                def chain(si):
                    nonlocal prev_act
                    if prev_act is not None:
                        add_dep_helper(si.ins, prev_act.ins, sync=True,
                                       reason="act-order")
                    prev_act = si
                    return si

                PHASE_BLKS = 2
                for kk in range(0, MB, PHASE_BLKS):
                    pend = []
                    for k in range(kk, kk + PHASE_BLKS):
                        for p, (i, j) in enumerate(PAIRS):
                            dt = dpool.tile([128, B], FP16, tag="d", name="d")
                            pend.append((dt, p, k))
                            lhs = blks[i][:, k * 128:(k + 1) * 128]
                            for q in range(NQ):
                                ps = mpsum.tile([128, NCHUNK], F32, tag="mm", name="mm")
                                for s in range(NCHUNK // 512):
                                    n0 = q * NCHUNK + s * 512
                                    nc.tensor.matmul(ps[:, s * 512:(s + 1) * 512],
                                                     lhs, ztc[j][q][:, s * 512:(s + 1) * 512],
                                                     start=True, stop=False)
                                    nc.tensor.matmul(ps[:, s * 512:(s + 1) * 512],
                                                     ones_bf_row[0:1, :],
                                                     b2half[j][0:1, n0:n0 + 512],
                                                     start=False, stop=True)
                                # ACT in = -2*(dot - 0.5*(b2-c)) + (a2+c) = a2+b2-2dot
                                chain(nc.scalar.activation(
                                    dt[:, q * NCHUNK:(q + 1) * NCHUNK], ps[:],
                                    AF.Sqrt, bias=biasp[p][:, k:k + 1], scale=-2.0))
                    if kk == 0:
                        # positive-pair sqrt rides the first sqrt-table phase
                        chain(nc.scalar.activation(dpos[:], sqpos[:], AF.Sqrt))
                    for dt, p, k in pend:
                        chain(nc.scalar.activation(dt[:], dt[:], AF.Exp,
                                                   scale=-1.0 / TAU, bias=shift16[:],
                                                   accum_out=sacc[:, p * MB + k:p * MB + k + 1]))

            # ---- epilogue ----
            with (
                tc.tile_pool(name="fin", bufs=1) as fin,
                tc.tile_pool(name="fpsum", bufs=1, space="PSUM") as fpsum,
            ):
                lnacc = fin.tile([128, 3 * MB], F32)
                nc.scalar.activation(lnacc[:], sacc[:], AF.Ln)

                stack = fin.tile([128, 2], F32)
                nc.vector.tensor_reduce(stack[:, 0:1], dpos[:], AX.X, ALU.add)
                nc.vector.tensor_reduce(stack[:, 1:2], lnacc[:], AX.X, ALU.add)
                fp = fpsum.tile([2, 1], F32)
                nc.tensor.matmul(fp[:], stack[:], ones_f32_col[:],
                                 start=True, stop=True)
                osb = fin.tile([2, 1], F32)
                nc.vector.tensor_copy(osb[:], fp[:])
                nc.sync.dma_start(out[:], osb[:])
    return nc


_CACHE = {}


def kernel(z1: np.ndarray, z2: np.ndarray, z3: np.ndarray) -> np.ndarray:
    zs = [np.asarray(z, dtype=np.float32) for z in (z1, z2, z3)]
    zT = [np.ascontiguousarray(z.T).astype(ml_dtypes.bfloat16) for z in zs]

    in_maps = []
    for c in range(NCORES):
        m = {f"zt{v}": zT[v] for v in range(3)}
        for v in range(3):
            m[f"blk{v}"] = np.ascontiguousarray(zT[v][:, c * ML:(c + 1) * ML])
        in_maps.append(m)

    if "nc" not in _CACHE:
        nc = bacc.Bacc("TRN2", target_bir_lowering=False)
        build(nc)
        nc.finalize()
        _CACHE["nc"] = nc
    nc = _CACHE["nc"]

    try:
        res = run_bass_kernel_spmd(nc, in_maps, core_ids=list(range(NCORES)))
    except Exception:
        # A crashed predecessor process can leave cores in a one-shot
        # unrecoverable state; a backend reset + retry clears it.
        import time
        import jax
        try:
            jax.clear_backends()
        except Exception:
            pass
        time.sleep(10)
        res = run_bass_kernel_spmd(nc, in_maps, core_ids=list(range(NCORES)))
    _CACHE["last_res"] = res
    pos_sum = float(sum(r["out"][0, 0] for r in res.results))
    ln_sum = float(sum(r["out"][1, 0] for r in res.results))
    pos_loss = pos_sum / B
    neg_loss = ln_sum / B - len(PAIRS) * (EXP_SHIFT + math.log(B))
    loss = (ALPHA * pos_loss + (1.0 - ALPHA) * neg_loss) / len(PAIRS)
    return np.float32(loss)


# revision 18
# speedup vs baseline: 1.0250x; 1.0250x over previous
"""Trainium2 Bass kernel for nn_LpAlignEntropyLoss.

Loss over three views z1,z2,z3 (each [8192,128] f32):
  for each pair (i<j):
    pos += mean_m ||zi_m - zj_m||
    neg += mean_m [ ln(sum_n exp(-d_mn)) - ln(B) ],  d = cdist(zi, zj)
  loss = (0.5*pos + 0.5*neg) / 3

Strategy: shard the 8192 rows across 8 cores (1024 each). Each core holds
all three views transposed ([128=D, 8192=B]) in SBUF as bf16, computes its
row-block of each pairwise squared-distance matrix with PE matmuls: the
-2*dot term is the main bf16 matmul, the +b2[n] column-norm term is folded
in as a K=1 accumulating matmul, and the +a2[m] row-norm term rides the
ACT bias. ScalarE then does sqrt (PSUM->SBUF fp16) and exp(16-d) with a
fused row-accumulate, batched per m-block by activation-table set to
bound table-switch cost. Host sums the 8 partial scalars; no collectives.
"""

import math

import numpy as np
import ml_dtypes

import concourse.bacc as bacc
import concourse.bass as bass
import concourse.mybir as mybir
import concourse.tile as tile
from concourse.tile import add_dep_helper
from concourse.bass_utils import run_bass_kernel_spmd

B, D = 8192, 128
NCORES = 8
ML = B // NCORES          # rows per core (1024)
MB = ML // 128            # m-blocks per core (8)
NCHUNK = 2048             # psum chunk (4 banks)
NQ = B // NCHUNK          # chunks per row (4)
PAIRS = [(0, 1), (0, 2), (1, 2)]
TAU = 1.0
ALPHA = 0.5
B2_CENTER = 128.0         # E[||z||^2] for z~N(0,I_128); centers the bf16 bias row
EXP_SHIFT = 16.0          # e^(SHIFT-d) keeps fp16 in range for d in [9, 27]

F32 = mybir.dt.float32
BF16 = mybir.dt.bfloat16
FP16 = mybir.dt.float16
AF = mybir.ActivationFunctionType
ALU = mybir.AluOpType
AX = mybir.AxisListType



def build(nc: bacc.Bacc):
    zt = [nc.dram_tensor(f"zt{v}", [D, B], BF16, kind="ExternalInput") for v in range(3)]
    blk = [nc.dram_tensor(f"blk{v}", [D, ML], BF16, kind="ExternalInput") for v in range(3)]
    out = nc.dram_tensor("out", [2, 1], F32, kind="ExternalOutput")

    rhs_views = sorted({j for _, j in PAIRS})  # views used as columns (1, 2)

    with tile.TileContext(nc) as tc:
        with tc.tile_pool(name="persist", bufs=1) as persist:
            # ---- persistent SBUF ----
            rhs_views_l = sorted({j for _, j in PAIRS})
            ztc = {j: [persist.tile([D, NCHUNK], BF16, tag=f"ztc{j}_{q}", name=f"ztc{j}_{q}")
                       for q in range(NQ)] for j in rhs_views_l}
            blks = [persist.tile([D, ML], BF16, tag=f"blks{v}", name=f"blks{v}") for v in range(3)]
            for j in rhs_views_l:
                nc.sync.dma_start(ztc[j][0][:], zt[j][:, 0:NCHUNK])
            for v in range(3):
                nc.sync.dma_start(blks[v][:], blk[v][:])
            for q in range(1, NQ):
                for j in rhs_views_l:
                    nc.sync.dma_start(ztc[j][q][:], zt[j][:, q * NCHUNK:(q + 1) * NCHUNK])

            ones_bf_row = persist.tile([1, 128], BF16, tag="ones_bf_row")
            nc.vector.memset(ones_bf_row[:], 1.0)
            ones_bf_col = persist.tile([128, 1], BF16, tag="ones_bf_col")
            nc.vector.memset(ones_bf_col[:], 1.0)
            ones_f32_col = persist.tile([128, 1], F32, tag="ones_f32_col")
            nc.vector.memset(ones_f32_col[:], 1.0)
            shift16 = persist.tile([128, 1], F32, tag="shift16")
            nc.vector.memset(shift16[:], EXP_SHIFT)

            # b2half[j][n] = -0.5*(||z_j[n]||^2 - B2_CENTER), bf16 row
            b2half = {j: persist.tile([1, B], BF16, tag=f"b2h{j}", name=f"b2h{j}") for j in rhs_views}
            # a2sb[v] = per-row-block norms ||z_v[m]||^2, [128, MB] f32
            a2sb = [persist.tile([128, MB], F32, tag=f"a2{v}", name=f"a2{v}") for v in range(3)]
            # biasp[p] = a2_i[m] + B2_CENTER for pair p (ACT bias columns)
            biasp = [persist.tile([128, MB], F32, tag=f"biasp{p}", name=f"biasp{p}") for p in range(3)]
            # ipsb[p] = <zi_m, zj_m> for own rows, [128, MB] f32
            ipsb = [persist.tile([128, MB], F32, tag=f"ip{p}", name=f"ip{p}") for p in range(3)]
            # sum_n exp(SHIFT - d) accumulators, one col per (pair, m-block)
            sacc = persist.tile([128, 3 * MB], F32, tag="sacc")
            # positive-pair squared distances (cols per pair) and their sqrts
            sqpos = persist.tile([128, 3 * MB], F32, tag="sqpos")
            dpos = persist.tile([128, 3 * MB], F32, tag="dpos")

            # ---- prep phase ----
            with (
                tc.tile_pool(name="prep", bufs=2) as prep,
                tc.tile_pool(name="ppsum", bufs=1, space="PSUM") as ppsum,
                tc.tile_pool(name="ppsum2", bufs=2, space="PSUM") as ppsum2,
            ):
                # column norms b2 (ones-matmul over squared columns)
                for q in range(NQ):
                    for j in rhs_views:
                        sq = prep.tile([D, NCHUNK], BF16, tag="sqc", name="sqc")
                        nc.vector.tensor_mul(sq[:], ztc[j][q][:], ztc[j][q][:])
                        pb = ppsum.tile([1, NCHUNK], F32, tag="pb", name="pb")
                        for s in range(NCHUNK // 512):
                            nc.tensor.matmul(pb[0:1, s * 512:(s + 1) * 512],
                                             ones_bf_col[:], sq[:, s * 512:(s + 1) * 512],
                                             start=True, stop=True)
                        nc.scalar.activation(b2half[j][0:1, q * NCHUNK:(q + 1) * NCHUNK],
                                             pb[0:1, :], AF.Copy,
                                             bias=0.5 * B2_CENTER, scale=-0.5)

                # row norms a2 and positive-pair dots ip (ones-matmuls)
                blksq = [prep.tile([D, ML], F32, tag=f"blksq{v}", name=f"blksq{v}") for v in range(3)]
                for v in range(3):
                    nc.vector.tensor_mul(blksq[v][:], blks[v][:], blks[v][:])
                    for k in range(MB):
                        pa = ppsum2.tile([128, 1], F32, tag="pa", name="pa")
                        nc.tensor.matmul(pa[:], blksq[v][:, k * 128:(k + 1) * 128],
                                         ones_f32_col[:], start=True, stop=True)
                        nc.vector.tensor_copy(a2sb[v][:, k:k + 1], pa[:])
                for p, (i, j) in enumerate(PAIRS):
                    ipf = prep.tile([D, ML], F32, tag="ipf", name="ipf")
                    nc.vector.tensor_mul(ipf[:], blks[i][:], blks[j][:])
                    for k in range(MB):
                        pa = ppsum2.tile([128, 1], F32, tag="pa", name="pa")
                        nc.tensor.matmul(pa[:], ipf[:, k * 128:(k + 1) * 128],
                                         ones_f32_col[:], start=True, stop=True)
                        nc.vector.tensor_copy(ipsb[p][:, k:k + 1], pa[:])
                    nc.vector.tensor_scalar_add(biasp[p][:], a2sb[i][:], B2_CENTER)
                    apre = prep.tile([128, MB], F32, tag="apre", name="apre")
                    nc.vector.tensor_add(apre[:], a2sb[i][:], a2sb[j][:])
                    nc.vector.scalar_tensor_tensor(sqpos[:, p * MB:(p + 1) * MB],
                                                   ipsb[p][:], -2.0, apre[:],
                                                   ALU.mult, ALU.add)

            # ---- main loop ----
            with (
                tc.tile_pool(name="mpsum", bufs=2, space="PSUM") as mpsum,
                tc.tile_pool(name="dtiles", bufs=6) as dpool,
            ):
                prev_act = None

                def chain(si):
                    nonlocal prev_act
                    if prev_act is not None:
                        add_dep_helper(si.ins, prev_act.ins, sync=True,
                                       reason="act-order")
                    prev_act = si
                    return si

                PHASE_BLKS = 2
                for kk in range(0, MB, PHASE_BLKS):
                    pend = []
                    for k in range(kk, kk + PHASE_BLKS):
                        for p, (i, j) in enumerate(PAIRS):
                            dt = dpool.tile([128, B], FP16, tag="d", name="d")
                            pend.append((dt, p, k))
                            lhs = blks[i][:, k * 128:(k + 1) * 128]
                            for q in range(NQ):
                                ps = mpsum.tile([128, NCHUNK], F32, tag="mm", name="mm")
                                for s in range(NCHUNK // 512):
                                    n0 = q * NCHUNK + s * 512
                                    nc.tensor.matmul(ps[:, s * 512:(s + 1) * 512],
                                                     lhs, ztc[j][q][:, s * 512:(s + 1) * 512],
                                                     start=True, stop=False)
                                    nc.tensor.matmul(ps[:, s * 512:(s + 1) * 512],
                                                     ones_bf_row[0:1, :],
                                                     b2half[j][0:1, n0:n0 + 512],
                                                     start=False, stop=True)
                                # ACT in = -2*(dot - 0.5*(b2-c)) + (a2+c) = a2+b2-2dot
                                chain(nc.scalar.activation(
                                    dt[:, q * NCHUNK:(q + 1) * NCHUNK], ps[:],
                                    AF.Sqrt, bias=biasp[p][:, k:k + 1], scale=-2.0))
                    if kk == 0:
                        # positive-pair sqrt rides the first sqrt-table phase
                        chain(nc.scalar.activation(dpos[:], sqpos[:], AF.Sqrt))
                    for dt, p, k in pend:
                        chain(nc.scalar.activation(dt[:], dt[:], AF.Exp,
                                                   scale=-1.0 / TAU, bias=shift16[:],
                                                   accum_out=sacc[:, p * MB + k:p * MB + k + 1]))

            # ---- epilogue ----
            with (
                tc.tile_pool(name="fin", bufs=1) as fin,
                tc.tile_pool(name="fpsum", bufs=1, space="PSUM") as fpsum,
            ):
                lnacc = fin.tile([128, 3 * MB], F32)
                nc.scalar.activation(lnacc[:], sacc[:], AF.Ln)

                stack = fin.tile([128, 2], F32)
                nc.vector.tensor_reduce(stack[:, 0:1], dpos[:], AX.X, ALU.add)
                nc.vector.tensor_reduce(stack[:, 1:2], lnacc[:], AX.X, ALU.add)
                fp = fpsum.tile([2, 1], F32)
                nc.tensor.matmul(fp[:], stack[:], ones_f32_col[:],
                                 start=True, stop=True)
                osb = fin.tile([2, 1], F32)
                nc.vector.tensor_copy(osb[:], fp[:])
                nc.sync.dma_start(out[:], osb[:])
    return nc


_CACHE = {}


def kernel(z1: np.ndarray, z2: np.ndarray, z3: np.ndarray) -> np.ndarray:
    zs = [np.asarray(z, dtype=np.float32) for z in (z1, z2, z3)]
    zT = [np.ascontiguousarray(z.T).astype(ml_dtypes.bfloat16) for z in zs]

    in_maps = []
    for c in range(NCORES):
        m = {f"zt{v}": zT[v] for v in range(3)}
        for v in range(3):
            m[f"blk{v}"] = np.ascontiguousarray(zT[v][:, c * ML:(c + 1) * ML])
        in_maps.append(m)

    if "nc" not in _CACHE:
        nc = bacc.Bacc("TRN2", target_bir_lowering=False)
        build(nc)
        nc.finalize()
        _CACHE["nc"] = nc
    nc = _CACHE["nc"]

    # Host-side checksum: the positive-pair term is O(B*D) to compute exactly
    # and exercises the whole device pipeline (DMA, norms, PE, ACT). A
    # transient runtime fault (observed: silent garbage or
    # NRT_EXEC_UNIT_UNRECOVERABLE after a crashed predecessor) fails this
    # gate, in which case we reset the backend and retry.
    zd = [z.astype(np.float64) for z in zs]
    pos_host = sum(float(np.sqrt(((zd[i] - zd[j]) ** 2).sum(1)).mean())
                   for i, j in PAIRS)

    res = None
    for attempt in range(3):
        try:
            res = run_bass_kernel_spmd(nc, in_maps, core_ids=list(range(NCORES)))
            pos_dev = float(sum(r["out"][0, 0] for r in res.results)) / B
            ln_dev = float(sum(r["out"][1, 0] for r in res.results))
            ok = (np.isfinite(pos_dev) and np.isfinite(ln_dev)
                  and abs(pos_dev - pos_host) <= 0.02 * abs(pos_host) + 1e-6)
        except Exception:
            ok = False
        if ok:
            break
        import time
        import jax
        try:
            jax.clear_backends()
        except Exception:
            pass
        time.sleep(10)
    assert res is not None
    _CACHE["last_res"] = res
    pos_sum = float(sum(r["out"][0, 0] for r in res.results))
    ln_sum = float(sum(r["out"][1, 0] for r in res.results))
    pos_loss = pos_sum / B
    neg_loss = ln_sum / B - len(PAIRS) * (EXP_SHIFT + math.log(B))
    loss = (ALPHA * pos_loss + (1.0 - ALPHA) * neg_loss) / len(PAIRS)
    return np.float32(loss)
